# revision 5
# baseline (speedup 1.0000x reference)
"""Trainium2 Bass kernel for DiamondLayer.

Computes out[b, d] = mean(x[b, d:d+16, d+17:d+33]) for d in [0, 2016):
16x16 mean-pool windows sliding along the diagonal of each 2048x2048 matrix.

Sharding: pure data parallel over batch - 32 batches -> 8 cores x 4 batches.

Per-core design (raw bacc). Measured facts driving it:
  - Only the diagonal band cols [r+2, r+34) of row r is ever touched. The
    DMA floor is ~2048 row descriptors per matrix (128B each, ~11.5ns per
    descriptor per DMA engine, ~14 engines shared by both HWDGE queues).
    All 8 band DMAs (row-halves x 4 batches) are issued dependency-free up
    front on the SP (rows 0-7) and ACT (rows 8-15) queues.
  - tensor_tensor_scan and tensor_reduce(axis=X) only exist on DVE on
    NeuronCore V3; tensor_tensor runs on Pool too. Windows never span
    rows, so prefix sums may reset at any row boundary: per batch DVE runs
    two half scans (rows 0-7 at P[pb..], rows 8-15 fresh at P[pb+256..]).
  - D[p, 16t+m] = P[32t+m+16] - P[32t+m] (window row-sums, bf16, computed
    by Pool; batch 3's low half on DVE to shorten the tail). Two D blocks
    are reused across batches (WAR guarded by the PE semaphore), padded by
    three 242-wide zero zones written once by Pool as bt0 - bt0.
  - out[16q+u] = sum_w D[.] splits into a within-partition part A[q] and a
    next-partition part B[q+1]. The idle PE contracts the partition shift
    AND pre-folds even/odd w: four accumulating bf16 matmuls per batch
    (lhsT = I/256 or shift/256 fed as inputs; moving = strided D views)
    yield PSUM[126, 8u+w'] so the final DVE reduce reads only 128 elems.
    The reduce lands the finished rows in SBUF; SP/ACT DMA them out.
"""

import os
import sys

import numpy as np

for _p in ("/opt/trn_rl_repo",):
    if _p not in sys.path:
        sys.path.insert(0, _p)

B_FULL = 32
N_CORES = 8
B_PER_CORE = B_FULL // N_CORES  # 4
MAT = 2048
RS = MAT + 1  # 2049
MAT_ELEMS = MAT * MAT
ND = MAT - 32  # 2016
NQ = ND // 16  # 126 output groups
NP = NQ + 1  # 127 partitions (last is B-source only)
PPW = 2048  # ppall width: 4 slices of 512
DW = 1280  # ddall width: Z0 [0,242) D0 [242,498) Z1 [498,740) D1 [740,996) Ze [996,1238)
DOFF = [242, 740]  # D block starts (batch b uses block b%2)

LAST_EXEC_TIME_NS = None
_COMPILED = None


def _build():
    import concourse.bass as bass
    import concourse.bacc as bacc
    from concourse import mybir
    from contextlib import ExitStack

    f32 = mybir.dt.float32
    bf16 = mybir.dt.bfloat16
    add = mybir.AluOpType.add
    sub_op = mybir.AluOpType.subtract
    bypass = mybir.AluOpType.bypass
    X = mybir.AxisListType.X

    nc = bacc.Bacc("TRN2", target_bir_lowering=False, debug=False)
    x = nc.dram_tensor("x", [B_PER_CORE, MAT, MAT], f32, kind="ExternalInput")
    w1d = nc.dram_tensor("w1", [NP, 128], bf16, kind="ExternalInput")
    w2d = nc.dram_tensor("w2", [NP, 128], bf16, kind="ExternalInput")
    y = nc.dram_tensor("y", [B_PER_CORE, ND], f32, kind="ExternalOutput")

    def v(t, off, pat):
        return bass.AP(t, off, pat)

    with ExitStack() as ctx:
        B = B_PER_CORE
        e = ctx.enter_context
        bts = [e(nc.sbuf_tensor(f"bt{i}", [NP, 512], f32)) for i in range(B)]
        ppall = e(nc.sbuf_tensor("ppall", [NP, PPW], f32))
        ddall = e(nc.sbuf_tensor("ddall", [NP, DW], bf16))
        ros = [e(nc.sbuf_tensor(f"ro{i}", [NQ, 16], f32)) for i in range(B)]
        w1 = e(nc.sbuf_tensor("w1s", [NP, 128], bf16))
        w2 = e(nc.sbuf_tensor("w2s", [NP, 128], bf16))
        pms = [e(nc.psum_tensor(f"pm{i}", [NQ, 128], f32)) for i in range(B)]

        sA = [e(nc.semaphore(f"sA{i}")) for i in range(B)]  # band rows 0-7
        sB = [e(nc.semaphore(f"sB{i}")) for i in range(B)]  # band rows 8-15
        sW = e(nc.semaphore("sW"))
        sPp = e(nc.semaphore("sPp"))  # ppall P[0]-columns memset
        sZ = e(nc.semaphore("sZ"))  # zero zones
        sScH1 = e(nc.semaphore("sScH1"))
        sScH2 = e(nc.semaphore("sScH2"))
        sSubD = e(nc.semaphore("sSubD"))  # Pool subD b=0..2
        sSubD3 = e(nc.semaphore("sSubD3"))  # DVE subD b=3
        sSubP = e(nc.semaphore("sSubP"))
        sPe = e(nc.semaphore("sPe"))
        sRed = e(nc.semaphore("sRed"))
        sOut = e(nc.semaphore("sOut"))
        block = e(nc.Block(no_gpsimd_drain=True))

        band_pat = [[16 * RS, NP], [RS, 8], [1, 32]]
        mm_pat = [[DW, NP], [16, 16], [30, 8]]

        @block.sync
        def _(sync):
            sync.dma_start(
                v(w1, 0, [[128, NP], [1, 128]]),
                v(w1d, 0, [[128, NP], [1, 128]]),
            ).then_inc(sW, 16)
            # band rows t in [0,8): bt[p, 32t+j] = x[b, 16p+t, 16p+t+2+j]
            for b in range(B):
                sync.dma_start(
                    v(bts[b], 0, [[512, NP], [32, 8], [1, 32]]),
                    v(x, b * MAT_ELEMS + 2, band_pat),
                ).then_inc(sA[b], 16)
            for b in range(3):
                sync.wait_ge(sRed, b + 1)
                sync.dma_start(
                    v(y, b * ND, [[16, NQ], [1, 16]]),
                    v(ros[b], 0, [[16, NQ], [1, 16]]),
                ).then_inc(sOut, 16)
            sync.wait_ge(sOut, 16 * B)

        @block.scalar
        def _(scalar):
            scalar.dma_start(
                v(w2, 0, [[128, NP], [1, 128]]),
                v(w2d, 0, [[128, NP], [1, 128]]),
            ).then_inc(sW, 16)
            # band rows t in [8,16)
            for b in range(B):
                scalar.dma_start(
                    v(bts[b], 256, [[512, NP], [32, 8], [1, 32]]),
                    v(x, b * MAT_ELEMS + 2 + 8 * RS, band_pat),
                ).then_inc(sB[b], 16)
            scalar.wait_ge(sRed, B)
            scalar.dma_start(
                v(y, 3 * ND, [[16, NQ], [1, 16]]),
                v(ros[3], 0, [[16, NQ], [1, 16]]),
            ).then_inc(sOut, 16)

        def scan_h1(b):
            pb = 512 * b
            nc.vector.tensor_tensor_scan(
                out=v(ppall, pb + 1, [[PPW, NP], [1, 255]]),
                data0=v(bts[b], 0, [[512, NP], [1, 255]]),
                data1=v(bts[b], 0, [[512, NP], [1, 255]]),
                initial=0.0,
                op0=add,
                op1=bypass,
            ).then_inc(sScH1, 1)

        def scan_h2(b):
            pb = 512 * b
            nc.vector.tensor_tensor_scan(
                out=v(ppall, pb + 257, [[PPW, NP], [1, 255]]),
                data0=v(bts[b], 256, [[512, NP], [1, 255]]),
                data1=v(bts[b], 256, [[512, NP], [1, 255]]),
                initial=0.0,
                op0=add,
                op1=bypass,
            ).then_inc(sScH2, 1)

        def sub_lo(eng, b, sem):
            # D[16t+m] = P[32t+m+16] - P[32t+m], t in [0,8), bf16 out
            pb = 512 * b
            eng.tensor_tensor(
                out=v(ddall, DOFF[b % 2], [[DW, NP], [16, 8], [1, 16]]),
                in0=v(ppall, pb + 16, [[PPW, NP], [32, 8], [1, 16]]),
                in1=v(ppall, pb, [[PPW, NP], [32, 8], [1, 16]]),
                op=sub_op,
            ).then_inc(sem, 1)

        def sub_hi(b):
            pb = 512 * b
            nc.gpsimd.tensor_tensor(
                out=v(ddall, DOFF[b % 2] + 128, [[DW, NP], [16, 8], [1, 16]]),
                in0=v(ppall, pb + 256 + 16, [[PPW, NP], [32, 8], [1, 16]]),
                in1=v(ppall, pb + 256, [[PPW, NP], [32, 8], [1, 16]]),
                op=sub_op,
            ).then_inc(sSubP, 1)

        def red(b):
            nc.vector.reduce_sum(
                out=v(ros[b], 0, [[16, NQ], [1, 16]]),
                in_=v(pms[b], 0, [[128, NQ], [8, 16], [1, 8]]),
                axis=X,
            ).then_inc(sRed, 1)

        @block.vector
        def _(vector):
            # P[0]-columns of all 4 pp slices: cols 512b and 512b+256
            nc.vector.memset(
                v(ppall, 0, [[PPW, NP], [512, B], [256, 2], [1, 1]]), 0.0
            ).then_inc(sPp, 1)
            vector.wait_ge(sA[0], 16)
            scan_h1(0)
            vector.wait_ge(sB[0], 16)
            scan_h2(0)
            vector.wait_ge(sA[1], 16)
            scan_h1(1)
            vector.wait_ge(sB[1], 16)
            scan_h2(1)
            vector.wait_ge(sPe, 1)
            red(0)
            vector.wait_ge(sA[2], 16)
            scan_h1(2)
            vector.wait_ge(sB[2], 16)
            scan_h2(2)
            vector.wait_ge(sA[3], 16)
            scan_h1(3)
            vector.wait_ge(sB[3], 16)
            scan_h2(3)
            # batch 3 low-half sub on DVE (tail: parallel with Pool's sub_hi)
            vector.wait_ge(sPe, 2)  # D1 block reuse: PE group 1 done
            sub_lo(nc.vector, 3, sSubD3)
            vector.wait_ge(sPe, 2)
            red(1)
            vector.wait_ge(sPe, 3)
            red(2)
            vector.wait_ge(sPe, 4)
            red(3)

        @block.gpsimd
        def _(gpsimd):
            # zero zones as bt0 - bt0 (exact zeros; bt0 is finite x data)
            gpsimd.wait_ge(sA[0], 16)
            # Zend [996,1238) from rows 0-7 half
            nc.gpsimd.tensor_tensor(
                out=v(ddall, 996, [[DW, NP], [1, 242]]),
                in0=v(bts[0], 0, [[512, NP], [1, 242]]),
                in1=v(bts[0], 0, [[512, NP], [1, 242]]),
                op=sub_op,
            ).then_inc(sZ, 1)
            # Z0 [0,242)
            nc.gpsimd.tensor_tensor(
                out=v(ddall, 0, [[DW, NP], [1, 242]]),
                in0=v(bts[0], 0, [[512, NP], [1, 242]]),
                in1=v(bts[0], 0, [[512, NP], [1, 242]]),
                op=sub_op,
            ).then_inc(sZ, 1)
            # Z1 [498,740)
            nc.gpsimd.tensor_tensor(
                out=v(ddall, 498, [[DW, NP], [1, 242]]),
                in0=v(bts[0], 0, [[512, NP], [1, 242]]),
                in1=v(bts[0], 0, [[512, NP], [1, 242]]),
                op=sub_op,
            ).then_inc(sZ, 1)
            gpsimd.wait_ge(sPp, 1)
            for b in range(B):
                if b < 3:
                    gpsimd.wait_ge(sScH1, b + 1)
                    if b == 2:
                        gpsimd.wait_ge(sPe, 1)  # D0 reuse
                    sub_lo(nc.gpsimd, b, sSubD)
                gpsimd.wait_ge(sScH2, b + 1)
                if b == 3:
                    gpsimd.wait_ge(sPe, 2)  # D1 reuse
                sub_hi(b)

        @block.tensor
        def _(tensor):
            tensor.wait_ge(sW, 32)
            tensor.wait_ge(sZ, 3)
            for b in range(B):
                if b < 3:
                    tensor.wait_ge(sSubD, b + 1)
                else:
                    tensor.wait_ge(sSubD3, 1)
                tensor.wait_ge(sSubP, b + 1)
                O = DOFF[b % 2]
                # psum[q, 8u+w'] = sum of A/B terms, even+odd w folded,
                # scaled by 1/256 via the selection matrices
                nc.tensor.matmul(
                    v(pms[b], 0, [[128, NQ], [1, 128]]),
                    v(w1, 0, [[128, NP], [1, NQ]]),
                    v(ddall, O + 15, mm_pat),
                    start=True,
                    stop=False,
                )
                nc.tensor.matmul(
                    v(pms[b], 0, [[128, NQ], [1, 128]]),
                    v(w1, 0, [[128, NP], [1, NQ]]),
                    v(ddall, O + 30, mm_pat),
                    start=False,
                    stop=False,
                )
                nc.tensor.matmul(
                    v(pms[b], 0, [[128, NQ], [1, 128]]),
                    v(w2, 0, [[128, NP], [1, NQ]]),
                    v(ddall, O - 241, mm_pat),
                    start=False,
                    stop=False,
                )
                nc.tensor.matmul(
                    v(pms[b], 0, [[128, NQ], [1, 128]]),
                    v(w2, 0, [[128, NP], [1, NQ]]),
                    v(ddall, O - 226, mm_pat),
                    start=False,
                    stop=True,
                ).then_inc(sPe, 1)

    nc.compile()
    return nc


def _get_compiled():
    global _COMPILED
    if _COMPILED is None:
        _COMPILED = _build()
    return _COMPILED


def _weights():
    try:
        from ml_dtypes import bfloat16 as bf
    except ImportError:
        import jax.numpy as jnp

        bf = jnp.bfloat16
    w1 = np.zeros((NP, 128), dtype=np.float32)
    w1[np.arange(NQ), np.arange(NQ)] = 1.0 / 256.0
    w2 = np.zeros((NP, 128), dtype=np.float32)
    w2[np.arange(1, NP), np.arange(NQ)] = 1.0 / 256.0
    return w1.astype(bf), w2.astype(bf)


def kernel(x: np.ndarray) -> np.ndarray:
    global LAST_EXEC_TIME_NS
    from concourse.bass_utils import run_bass_kernel_spmd

    x = np.ascontiguousarray(np.asarray(x), dtype=np.float32)
    assert x.shape == (B_FULL, MAT, MAT), x.shape

    nc = _get_compiled()
    w1, w2 = _weights()
    in_maps = [
        {"x": x[i * B_PER_CORE : (i + 1) * B_PER_CORE], "w1": w1, "w2": w2}
        for i in range(N_CORES)
    ]
    trace = bool(int(os.environ.get("KERNEL_TRACE", "0")))
    if trace:
        # test-only: keep NTFF artifacts local instead of uploading
        from concourse import bass_utils as _bu

        _bu.upload_artifacts = lambda tmpdir: tmpdir
    res = run_bass_kernel_spmd(
        nc, in_maps, core_ids=list(range(N_CORES)), trace=trace
    )
    LAST_EXEC_TIME_NS = res.exec_time_ns
    out = np.concatenate([res.results[i]["y"] for i in range(N_CORES)], axis=0)
    return out.astype(np.float32)


# revision 10
# speedup vs baseline: 3.4486x; 3.4486x over previous
"""Trainium2 Bass kernel for DiamondLayer.

Computes out[b, d] = mean(x[b, d:d+16, d+17:d+33]) for d in [0, 2016):
16x16 mean-pool windows sliding along the diagonal of each 2048x2048 matrix.

Sharding: pure data parallel over batch - 32 batches -> 8 cores x 4 batches.

Per-core design (raw bacc). Measured facts driving it:
  - Only the diagonal band cols [r+2, r+34) of row r is ever touched, as
    one 128B descriptor per row. A DMA's descriptors are spread over the
    largest divisor of its PARTITION COUNT that is <= 16 DMA engines
    (127 partitions -> 1 engine -> 20us/transfer!), so band DMAs use 112
    partitions (16 engines); partitions 112-126 of all 4 batches ride in
    one early 15-partition tail DMA per queue. ~11.5ns per descriptor per
    engine; the engine pool is shared by both HWDGE queues.
  - tensor_tensor_scan and tensor_reduce(axis=X) only exist on DVE on
    NeuronCore V3; tensor_tensor runs on Pool too. Windows never span
    rows, so prefix sums may reset at any row boundary: per batch DVE runs
    two half scans (rows 0-7 at P[pb..], rows 8-15 fresh at P[pb+256..]).
    All 8 scans run before the 4 reduces (reduces gate only the output
    DMAs, scans gate everything).
  - D[p, 16t+m] = P[32t+m+16] - P[32t+m] (window row-sums, bf16, computed
    by Pool; batch 3's low half on DVE to shorten the tail). Two D blocks
    are reused across batches (WAR guarded by the PE semaphore), padded by
    three 242-wide zero zones written once by Pool as band - band.
  - out[16q+u] = sum_w D[.] splits into a within-partition part A[q] and a
    next-partition part B[q+1]. The idle PE contracts the partition shift
    AND pre-folds even/odd w: four accumulating bf16 matmuls per batch
    (lhsT = I/256 or shift/256 fed as inputs; moving = strided D views)
    yield PSUM[126, 8u+w'] so the final DVE reduce reads only 128 elems
    and lands the finished rows in SBUF; SP/ACT DMA them out.
"""

import os
import sys

import numpy as np

for _p in ("/opt/trn_rl_repo",):
    if _p not in sys.path:
        sys.path.insert(0, _p)

B_FULL = 32
N_CORES = 8
B_PER_CORE = B_FULL // N_CORES  # 4
MAT = 2048
RS = MAT + 1  # 2049
MAT_ELEMS = MAT * MAT
ND = MAT - 32  # 2016
NQ = ND // 16  # 126 output groups
NP = NQ + 1  # 127 partitions (last is B-source only)
MP = 112  # main-DMA partitions (16 DMA engines); tail: 15 partitions
BTW = 2048  # btall width: 4 slices of 512
PPW = 2048  # ppall width: 4 slices of 512
DW = 1280  # ddall: Z0 [0,242) D0 [242,498) Z1 [498,740) D1 [740,996) Ze [996,1238)
DOFF = [242, 740]  # D block starts (batch b uses block b%2)

LAST_EXEC_TIME_NS = None
_COMPILED = None


def _build():
    import concourse.bass as bass
    import concourse.bacc as bacc
    from concourse import mybir
    from contextlib import ExitStack

    f32 = mybir.dt.float32
    bf16 = mybir.dt.bfloat16
    add = mybir.AluOpType.add
    sub_op = mybir.AluOpType.subtract
    bypass = mybir.AluOpType.bypass
    X = mybir.AxisListType.X

    nc = bacc.Bacc("TRN2", target_bir_lowering=False, debug=False)
    x = nc.dram_tensor("x", [B_PER_CORE, MAT, MAT], f32, kind="ExternalInput")
    w1d = nc.dram_tensor("w1", [128, 128], bf16, kind="ExternalInput")
    w2d = nc.dram_tensor("w2", [128, 128], bf16, kind="ExternalInput")
    y = nc.dram_tensor("y", [B_PER_CORE, ND], f32, kind="ExternalOutput")

    def v(t, off, pat):
        return bass.AP(t, off, pat)

    with ExitStack() as ctx:
        B = B_PER_CORE
        e = ctx.enter_context
        btall = e(nc.sbuf_tensor("btall", [NP, BTW], f32))
        ppall = e(nc.sbuf_tensor("ppall", [NP, PPW], f32))
        ddall = e(nc.sbuf_tensor("ddall", [NP, DW], bf16))
        ros = [e(nc.sbuf_tensor(f"ro{i}", [NQ, 16], f32)) for i in range(B)]
        w1 = e(nc.sbuf_tensor("w1s", [128, 128], bf16))
        w2 = e(nc.sbuf_tensor("w2s", [128, 128], bf16))
        pms = [e(nc.psum_tensor(f"pm{i}", [NQ, 128], f32)) for i in range(B)]

        sA = [e(nc.semaphore(f"sA{i}")) for i in range(B)]  # rows 0-7 (2 DMAs)
        sB = [e(nc.semaphore(f"sB{i}")) for i in range(B)]  # rows 8-15 (2 DMAs)
        sW = e(nc.semaphore("sW"))
        sPp = e(nc.semaphore("sPp"))  # ppall P[0]-columns memset
        sZ = e(nc.semaphore("sZ"))  # zero zones
        sScH1 = e(nc.semaphore("sScH1"))
        sScH2 = e(nc.semaphore("sScH2"))
        sSubD = e(nc.semaphore("sSubD"))  # Pool subD b=0..2
        sSubD3 = e(nc.semaphore("sSubD3"))  # DVE subD b=3
        sSubP = e(nc.semaphore("sSubP"))
        sPe = e(nc.semaphore("sPe"))
        sRed = e(nc.semaphore("sRed"))
        sOut = e(nc.semaphore("sOut"))
        block = e(nc.Block(no_gpsimd_drain=True))

        main_src = [[16 * RS, MP], [RS, 8], [1, 32]]
        tail_src = [[16 * RS, NP - MP], [RS, 8], [1, 32]]
        mm_pat = [[DW, NP], [16, 16], [30, 8]]

        def band(eng, b, hi, sem):
            # btall[p, 512b+32t+j] = x[b, 16p+t+8*hi, 16p+t+8*hi+2+j]
            off = b * MAT_ELEMS + 2 + 8 * RS * hi
            eng.dma_start(
                v(btall, 512 * b + 256 * hi, [[BTW, MP], [32, 8], [1, 32]]),
                v(x, off, main_src),
            ).then_inc(sem, 16)
            eng.dma_start(
                v(
                    btall,
                    MP * BTW + 512 * b + 256 * hi,
                    [[BTW, NP - MP], [32, 8], [1, 32]],
                ),
                v(x, off + MP * 16 * RS, tail_src),
            ).then_inc(sem, 16)

        @block.sync
        def _(sync):
            # rows 0-7 halves
            band(sync, 0, 0, sA[0])
            band(sync, 1, 0, sA[1])
            sync.dma_start(
                v(w1, 0, [[128, 128], [1, 128]]),
                v(w1d, 0, [[128, 128], [1, 128]]),
            ).then_inc(sW, 16)
            band(sync, 2, 0, sA[2])
            band(sync, 3, 0, sA[3])
            for b in (0, 2):
                sync.wait_ge(sRed, b + 1)
                sync.dma_start(
                    v(y, b * ND, [[16, NQ], [1, 16]]),
                    v(ros[b], 0, [[16, NQ], [1, 16]]),
                ).then_inc(sOut, 16)
            sync.wait_ge(sOut, 16 * B)

        @block.scalar
        def _(scalar):
            # rows 8-15 halves
            band(scalar, 0, 1, sB[0])
            band(scalar, 1, 1, sB[1])
            scalar.dma_start(
                v(w2, 0, [[128, 128], [1, 128]]),
                v(w2d, 0, [[128, 128], [1, 128]]),
            ).then_inc(sW, 16)
            band(scalar, 2, 1, sB[2])
            band(scalar, 3, 1, sB[3])
            for b in (1, 3):
                scalar.wait_ge(sRed, b + 1)
                scalar.dma_start(
                    v(y, b * ND, [[16, NQ], [1, 16]]),
                    v(ros[b], 0, [[16, NQ], [1, 16]]),
                ).then_inc(sOut, 16)

        def scan_h1(b):
            pb = 512 * b
            nc.vector.tensor_tensor_scan(
                out=v(ppall, pb + 1, [[PPW, NP], [1, 255]]),
                data0=v(btall, pb, [[BTW, NP], [1, 255]]),
                data1=v(btall, pb, [[BTW, NP], [1, 255]]),
                initial=0.0,
                op0=add,
                op1=bypass,
            ).then_inc(sScH1, 1)

        def scan_h2(b):
            pb = 512 * b
            nc.vector.tensor_tensor_scan(
                out=v(ppall, pb + 257, [[PPW, NP], [1, 255]]),
                data0=v(btall, pb + 256, [[BTW, NP], [1, 255]]),
                data1=v(btall, pb + 256, [[BTW, NP], [1, 255]]),
                initial=0.0,
                op0=add,
                op1=bypass,
            ).then_inc(sScH2, 1)

        def sub_lo(eng, b, sem):
            # D[16t+m] = P[32t+m+16] - P[32t+m], t in [0,8), bf16 out
            pb = 512 * b
            eng.tensor_tensor(
                out=v(ddall, DOFF[b % 2], [[DW, NP], [16, 8], [1, 16]]),
                in0=v(ppall, pb + 16, [[PPW, NP], [32, 8], [1, 16]]),
                in1=v(ppall, pb, [[PPW, NP], [32, 8], [1, 16]]),
                op=sub_op,
            ).then_inc(sem, 1)

        def sub_hi(b):
            pb = 512 * b
            nc.gpsimd.tensor_tensor(
                out=v(ddall, DOFF[b % 2] + 128, [[DW, NP], [16, 8], [1, 16]]),
                in0=v(ppall, pb + 256 + 16, [[PPW, NP], [32, 8], [1, 16]]),
                in1=v(ppall, pb + 256, [[PPW, NP], [32, 8], [1, 16]]),
                op=sub_op,
            ).then_inc(sSubP, 1)

        def red(b):
            nc.vector.reduce_sum(
                out=v(ros[b], 0, [[16, NQ], [1, 16]]),
                in_=v(pms[b], 0, [[128, NQ], [8, 16], [1, 8]]),
                axis=X,
            ).then_inc(sRed, 1)

        @block.vector
        def _(vector):
            # P[0]-columns of all 4 pp slices: cols 512b and 512b+256
            nc.vector.memset(
                v(ppall, 0, [[PPW, NP], [512, B], [256, 2], [1, 1]]), 0.0
            ).then_inc(sPp, 1)
            for b in range(B):
                vector.wait_ge(sA[b], 32)
                scan_h1(b)
                vector.wait_ge(sB[b], 32)
                scan_h2(b)
            # batch 3 low-half sub on DVE (tail: parallel with Pool's sub_hi)
            vector.wait_ge(sPe, 2)  # D1 block reuse: PE group 1 done
            sub_lo(nc.vector, 3, sSubD3)
            vector.wait_ge(sPe, 1)
            red(0)
            vector.wait_ge(sPe, 2)
            red(1)
            vector.wait_ge(sPe, 3)
            red(2)
            vector.wait_ge(sPe, 4)
            red(3)

        @block.gpsimd
        def _(gpsimd):
            # zero zones as band - band (exact zeros; band is finite x data)
            gpsimd.wait_ge(sA[0], 32)
            for zoff in (0, 498):  # Z0, Z1
                nc.gpsimd.tensor_tensor(
                    out=v(ddall, zoff, [[DW, NP], [1, 242]]),
                    in0=v(btall, 0, [[BTW, NP], [1, 242]]),
                    in1=v(btall, 0, [[BTW, NP], [1, 242]]),
                    op=sub_op,
                ).then_inc(sZ, 1)
            gpsimd.wait_ge(sPp, 1)
            gpsimd.wait_ge(sScH1, 1)
            sub_lo(nc.gpsimd, 0, sSubD)
            gpsimd.wait_ge(sScH2, 1)
            sub_hi(0)
            # Zend
            nc.gpsimd.tensor_tensor(
                out=v(ddall, 996, [[DW, NP], [1, 242]]),
                in0=v(btall, 0, [[BTW, NP], [1, 242]]),
                in1=v(btall, 0, [[BTW, NP], [1, 242]]),
                op=sub_op,
            ).then_inc(sZ, 1)
            for b in (1, 2):
                gpsimd.wait_ge(sScH1, b + 1)
                if b == 2:
                    gpsimd.wait_ge(sPe, 1)  # D0 reuse
                sub_lo(nc.gpsimd, b, sSubD)
                gpsimd.wait_ge(sScH2, b + 1)
                sub_hi(b)
            gpsimd.wait_ge(sScH2, 4)
            gpsimd.wait_ge(sPe, 2)  # D1 reuse
            sub_hi(3)

        @block.tensor
        def _(tensor):
            tensor.wait_ge(sW, 32)
            for b in range(B):
                tensor.wait_ge(sZ, 2 if b == 0 else 3)
                if b < 3:
                    tensor.wait_ge(sSubD, b + 1)
                else:
                    tensor.wait_ge(sSubD3, 1)
                tensor.wait_ge(sSubP, b + 1)
                O = DOFF[b % 2]
                # psum[q, 8u+w'] = sum of A/B terms, even+odd w folded,
                # scaled by 1/256 via the selection matrices
                nc.tensor.matmul(
                    v(pms[b], 0, [[128, NQ], [1, 128]]),
                    v(w1, 0, [[128, NP], [1, NQ]]),
                    v(ddall, O + 15, mm_pat),
                    start=True,
                    stop=False,
                )
                nc.tensor.matmul(
                    v(pms[b], 0, [[128, NQ], [1, 128]]),
                    v(w1, 0, [[128, NP], [1, NQ]]),
                    v(ddall, O + 30, mm_pat),
                    start=False,
                    stop=False,
                )
                nc.tensor.matmul(
                    v(pms[b], 0, [[128, NQ], [1, 128]]),
                    v(w2, 0, [[128, NP], [1, NQ]]),
                    v(ddall, O - 241, mm_pat),
                    start=False,
                    stop=False,
                )
                nc.tensor.matmul(
                    v(pms[b], 0, [[128, NQ], [1, 128]]),
                    v(w2, 0, [[128, NP], [1, NQ]]),
                    v(ddall, O - 226, mm_pat),
                    start=False,
                    stop=True,
                ).then_inc(sPe, 1)

    nc.compile()
    return nc


def _get_compiled():
    global _COMPILED
    if _COMPILED is None:
        _COMPILED = _build()
    return _COMPILED


def _weights():
    try:
        from ml_dtypes import bfloat16 as bf
    except ImportError:
        import jax.numpy as jnp

        bf = jnp.bfloat16
    w1 = np.zeros((128, 128), dtype=np.float32)
    w1[np.arange(NQ), np.arange(NQ)] = 1.0 / 256.0
    w2 = np.zeros((128, 128), dtype=np.float32)
    w2[np.arange(1, NP), np.arange(NQ)] = 1.0 / 256.0
    return w1.astype(bf), w2.astype(bf)


def kernel(x: np.ndarray) -> np.ndarray:
    global LAST_EXEC_TIME_NS
    from concourse.bass_utils import run_bass_kernel_spmd

    x = np.ascontiguousarray(np.asarray(x), dtype=np.float32)
    assert x.shape == (B_FULL, MAT, MAT), x.shape

    nc = _get_compiled()
    w1, w2 = _weights()
    in_maps = [
        {"x": x[i * B_PER_CORE : (i + 1) * B_PER_CORE], "w1": w1, "w2": w2}
        for i in range(N_CORES)
    ]
    trace = bool(int(os.environ.get("KERNEL_TRACE", "0")))
    if trace:
        # test-only: keep NTFF artifacts local instead of uploading
        from concourse import bass_utils as _bu

        _bu.upload_artifacts = lambda tmpdir: tmpdir
    res = run_bass_kernel_spmd(
        nc, in_maps, core_ids=list(range(N_CORES)), trace=trace
    )
    LAST_EXEC_TIME_NS = res.exec_time_ns
    out = np.concatenate([res.results[i]["y"] for i in range(N_CORES)], axis=0)
    return out.astype(np.float32)
